# revision 18
# baseline (speedup 1.0000x reference)
"""Trainium2 Bass kernel for chunked delta-rule attention (DeltaNet-style).

Reference computation (per chunk of C=64 rows, sequential state S [d,d]):
    T = I - strict_lower(kb k^T);  W = T kb;  U = T vb
    u = U - W S;  o = lower(q k^T) u + q S;  S += k^T u

Parallelization: the state update is affine, S_i = (I+G_i) S_{i-1} + b_i with
    G_i = -k_i^T W_i,  b_i = k_i^T U_i   (chunk-local, state-independent).
Each of 8 cores handles 128 chunks: it runs two local chains from zero state
(A = prod(I+G), B = inhomogeneous part), stores per-chunk (U, At, Rt, Zt, B_prev),
AllGathers (A^T, B) across cores, computes its true incoming state S0_c with a
masked uniform 7-step prefix, then emits o = At^T U + Rt^T B_prev + Zt^T S0
entirely in PSUM and DMAs straight to HBM.

All matmul operands are bf16 (fp32 PSUM accumulation). The reference blows up
(~3000x per chunk; NaN from row ~704 on), so any implementation's late-sequence
values are chaos-dominated; bf16 matches the f32 reference to ~1.5e-2 relative
in the finite region (validated against numpy prototype).
"""

import os
import numpy as np
import ml_dtypes

BF = ml_dtypes.bfloat16
D, C, NCORES = 128, 64, 8
# PC overridable for small-scale debugging (KERNEL_PC env var)
PC = int(os.environ.get("KERNEL_PC", "128"))   # chunks per core
NCHUNKS = PC * NCORES
L = NCHUNKS * C
ROWS = PC * C                   # rows per core
NB = 8                          # chunks per stream block

_CACHE = {}
LAST_RESULTS = None  # BassKernelResults from the most recent run (for profiling)


def _build_nc():
    import concourse.mybir as mybir
    import concourse.tile as tile
    from concourse import bacc

    dt = mybir.dt
    nc = bacc.Bacc(
        "TRN2",
        target_bir_lowering=False,
        debug=False,
        enable_asserts=False,
        num_devices=NCORES,
    )

    kkq_d = nc.dram_tensor("kkq", [D, PC * 192], dt.bfloat16, kind="ExternalInput").ap()
    knat_d = nc.dram_tensor("knat", [ROWS, D], dt.bfloat16, kind="ExternalInput").ap()
    vkn_d = nc.dram_tensor("vkn", [ROWS, 2 * D], dt.bfloat16, kind="ExternalInput").ap()
    maskf_d = nc.dram_tensor("maskf", [D, NCORES * D], dt.uint8, kind="ExternalInput").ap()
    out_d = nc.dram_tensor("out", [ROWS, D], dt.float32, kind="ExternalOutput").ap()

    knat_v = knat_d.rearrange("(n p) d -> p n d", p=C)      # [64, PC, 128]
    vkn_v = vkn_d.rearrange("(n p) d -> p n d", p=C)        # [64, PC, 256]
    out_v = out_d.rearrange("(n p) d -> p n d", p=C)        # [64, PC, 128]

    dbg = {}
    if int(os.environ.get("KERNEL_DEBUG", "0")):
        for nm, shape in [("dbg_ball", [D, PC * D]), ("dbg_zt", [D, PC * C]),
                          ("dbg_rt", [D, PC * C]), ("dbg_s0", [D, D]),
                          ("dbg_gath", [NCORES * D, 2 * D])]:
            dbg[nm] = nc.dram_tensor(nm, shape, dt.bfloat16, kind="ExternalOutput").ap()

    with tile.TileContext(nc) as tc:
        _emit(tc, nc, mybir, kkq_d, knat_v, vkn_v, maskf_d, out_v, dbg)
    nc.compile()
    return nc


def _emit(tc, nc, mybir, kkq_d, knat_v, vkn_v, maskf_d, out_v, dbg=None):
    from contextlib import ExitStack

    dt = mybir.dt
    f32, bf16 = dt.float32, dt.bfloat16

    with ExitStack() as ctx:
        singles = ctx.enter_context(tc.tile_pool(name="singles", bufs=1))
        persist = ctx.enter_context(tc.tile_pool(name="persist", bufs=1))

        # ---- constants ----
        eye128b = singles.tile([D, D], bf16)
        nc.gpsimd.memset(eye128b, 0.0)
        nc.gpsimd.affine_select(
            out=eye128b, in_=eye128b, compare_op=mybir.AluOpType.not_equal,
            fill=1.0, base=0, pattern=[[-1, D]], channel_multiplier=1,
        )
        # masks replicated x4 (per PA-group of 4 chunks), [64, 4, 64]
        maskneg = singles.tile([C, 4, C], f32)   # -1 strictly-upper
        nc.gpsimd.memset(maskneg, 0.0)
        nc.gpsimd.affine_select(
            out=maskneg, in_=maskneg, compare_op=mybir.AluOpType.is_ge,
            fill=-1.0, base=0, pattern=[[0, 4], [-1, C]], channel_multiplier=1,
        )
        maskup = singles.tile([C, 4, C], f32)    # +1 upper incl diag
        nc.gpsimd.memset(maskup, 1.0)
        nc.gpsimd.affine_select(
            out=maskup, in_=maskup, compare_op=mybir.AluOpType.is_ge,
            fill=0.0, base=0, pattern=[[0, 4], [1, C]], channel_multiplier=-1,
        )
        eyerep = singles.tile([C, 4, C], bf16)   # identity x4
        nc.gpsimd.memset(eyerep, 0.0)
        nc.gpsimd.affine_select(
            out=eyerep, in_=eyerep, compare_op=mybir.AluOpType.not_equal,
            fill=1.0, base=0, pattern=[[0, 4], [-1, C]], channel_multiplier=1,
        )
        maskf_sb = singles.tile([D, NCORES * D], dt.uint8)
        nc.sync.dma_start(out=maskf_sb, in_=maskf_d)

        # ---- persistent per-chunk storage ----
        UW = persist.tile([C, PC * 256], bf16)     # per chunk: [U | Wn]
        Att = persist.tile([C, PC * C], bf16)      # A^T (masked scores)
        Rt = persist.tile([D, PC * C], bf16)       # R^T
        Zt = persist.tile([D, PC * C], bf16)       # (R A_prev)^T
        Ball = persist.tile([D, PC * D], bf16)     # B prefix after each chunk

        apool = ctx.enter_context(tc.tile_pool(name="apool", bufs=3))
        A_prev = eye128b

        with tc.tile_pool(name="chain_ps", bufs=1, space="PSUM") as chain_ps, \
             tc.tile_pool(name="pa_ps", bufs=2, space="PSUM") as pa_ps, \
             tc.tile_pool(name="uw_ps", bufs=1, space="PSUM") as uw_ps, \
             tc.tile_pool(name="gr_ps", bufs=2, space="PSUM") as gr_ps, \
             tc.tile_pool(name="z_ps", bufs=1, space="PSUM") as z_ps, \
             tc.tile_pool(name="streams", bufs=3) as streams, \
             tc.tile_pool(name="blocktmp", bufs=2) as blocktmp:

            chA = chain_ps.tile([D, D], mybir.dt.float32, tag="chA")
            chB = chain_ps.tile([D, D], mybir.dt.float32, tag="chB")

            for b in range(PC // NB):
                c0 = b * NB
                kkq_t = streams.tile([D, NB * 192], bf16, tag="kkq")
                nc.sync.dma_start(out=kkq_t, in_=kkq_d[:, c0 * 192:(c0 + NB) * 192])
                knat_t = streams.tile([C, NB, D], bf16, tag="knat")
                nc.sync.dma_start(out=knat_t, in_=knat_v[:, c0:c0 + NB, :])
                vkn_t = streams.tile([C, NB, 2 * D], bf16, tag="vkn")
                nc.sync.dma_start(out=vkn_t, in_=vkn_v[:, c0:c0 + NB, :])

                kkq3 = kkq_t.rearrange("p (i x) -> p i x", x=192)
                gt_t = blocktmp.tile([D, NB * D], bf16, tag="gt")
                zp = z_ps.tile([D, NB * C], mybir.dt.float32, tag="zp")

                # --- PA groups of 4 chunks: scores + masks ---
                ttf_list = []
                for g in range(NB // 4):
                    i0 = g * 4
                    pa = pa_ps.tile([C, 4 * 2 * C], mybir.dt.float32, tag="pa")
                    for j in range(4):
                        i = i0 + j
                        nc.tensor.matmul(
                            pa[:, j * 128:(j + 1) * 128],
                            lhsT=kkq3[:, i, 0:64], rhs=kkq3[:, i, 64:192],
                            start=True, stop=True,
                        )
                    pa3 = pa.rearrange("p (j x) -> p j x", x=128)
                    ttn = blocktmp.tile([C, 4, C], bf16, tag="ttn")
                    nc.vector.tensor_mul(ttn, pa3[:, :, 0:64], maskneg)
                    ttf = blocktmp.tile([C, 4, C], bf16, tag="ttf")
                    nc.vector.tensor_add(ttf, ttn, eyerep)
                    ttf_list.append(ttf)
                    att_s = Att[:, (c0 + i0) * C:(c0 + i0 + 4) * C]
                    nc.vector.tensor_mul(
                        att_s.rearrange("p (j x) -> p j x", x=C),
                        pa3[:, :, 64:128], maskup,
                    )

                # --- UW groups of 2 chunks ---
                for g in range(NB // 2):
                    i0 = g * 2
                    uw = uw_ps.tile([C, 2 * 256], mybir.dt.float32, tag="uw")
                    for j in range(2):
                        i = i0 + j
                        ttf = ttf_list[i // 4]
                        nc.tensor.matmul(
                            uw[:, j * 256:(j + 1) * 256],
                            lhsT=ttf[:, i % 4, :], rhs=vkn_t[:, i, :],
                            start=True, stop=True,
                        )
                    nc.scalar.copy(UW[:, (c0 + i0) * 256:(c0 + i0 + 2) * 256], uw)

                # --- GR groups of 2 chunks: Gt = Wn^T k, Rtpre = Wn^T At ---
                for g in range(NB // 2):
                    i0 = g * 2
                    gr = gr_ps.tile([D, 2 * 192], mybir.dt.float32, tag="gr")
                    for j in range(2):
                        i = i0 + j
                        ci = c0 + i
                        wn = UW[:, ci * 256 + 128:(ci + 1) * 256]
                        nc.tensor.matmul(gr[:, j * 192:j * 192 + 128],
                                         lhsT=wn, rhs=knat_t[:, i, :],
                                         start=True, stop=True)
                        nc.tensor.matmul(gr[:, j * 192 + 128:(j + 1) * 192],
                                         lhsT=wn, rhs=Att[:, ci * C:(ci + 1) * C],
                                         start=True, stop=True)
                    gr3 = gr.rearrange("p (j x) -> p j x", x=192)
                    nc.scalar.copy(
                        gt_t.rearrange("p (i x) -> p i x", x=D)[:, i0:i0 + 2, :],
                        gr3[:, :, 0:128],
                    )
                    # Rt = qT + Wn^T At
                    rt_s = Rt[:, (c0 + i0) * C:(c0 + i0 + 2) * C]
                    nc.vector.tensor_add(
                        rt_s.rearrange("p (j x) -> p j x", x=C),
                        gr3[:, :, 128:192],
                        kkq3[:, i0:i0 + 2, 128:192],
                    )

                # --- per-chunk: Zt + chains ---
                gt3 = gt_t.rearrange("p (i x) -> p i x", x=D)
                for i in range(NB):
                    ci = c0 + i
                    # Zt_i = A_{i-1}^T R_i^T  (lhsT = A_prev)
                    nc.tensor.matmul(zp[:, i * C:(i + 1) * C],
                                     lhsT=A_prev, rhs=Rt[:, ci * C:(ci + 1) * C],
                                     start=True, stop=True)
                    # chain A (closed group): psum = I A_{i-1} + G_i A_{i-1}
                    nc.tensor.matmul(chA, lhsT=eye128b, rhs=A_prev,
                                     start=True, stop=False)
                    nc.tensor.matmul(chA, lhsT=gt3[:, i, :], rhs=A_prev,
                                     start=False, stop=True)
                    A_new = apool.tile([D, D], bf16, tag="achain")
                    nc.scalar.copy(A_new, chA)
                    # chain B (closed group): psum = B_{i-1} + G_i B_{i-1} + k_i^T U_i
                    if ci > 0:
                        bprev = Ball[:, (ci - 1) * D:ci * D]
                        nc.tensor.matmul(chB, lhsT=eye128b, rhs=bprev,
                                         start=True, stop=False)
                        nc.tensor.matmul(chB, lhsT=gt3[:, i, :], rhs=bprev,
                                         start=False, stop=False)
                    nc.tensor.matmul(chB, lhsT=knat_t[:, i, :],
                                     rhs=UW[:, ci * 256:ci * 256 + 128],
                                     start=(ci == 0), stop=True)
                    nc.scalar.copy(Ball[:, ci * D:(ci + 1) * D], chB)
                    A_prev = A_new

                nc.scalar.copy(Zt[:, c0 * C:(c0 + NB) * C], zp)

        # ---- combine: AllGather (A^T | B), masked prefix -> S0 ----
        with tc.tile_pool(name="comb_ps", bufs=1, space="PSUM") as comb_ps, \
             tc.tile_pool(name="comb", bufs=2) as comb, \
             tc.tile_pool(name="dram", bufs=1, space="DRAM") as dram:

            tp = comb_ps.tile([D, D], bf16, tag="tp")
            nc.tensor.transpose(tp, A_prev, eye128b)
            tps = comb.tile([D, D], bf16, tag="tps")
            nc.scalar.copy(tps, tp)

            bounce = dram.tile([D, 2 * D], bf16)
            gath = dram.tile([NCORES * D, 2 * D], bf16)
            nc.sync.dma_start(out=bounce[:, 0:D], in_=tps)
            nc.sync.dma_start(out=bounce[:, D:2 * D], in_=Ball[:, (PC - 1) * D:PC * D])
            nc.gpsimd.collective_compute(
                "AllGather", mybir.AluOpType.bypass,
                replica_groups=[list(range(NCORES))],
                ins=[bounce.opt()], outs=[gath.opt()],
            )
            gsb = comb.tile([D, NCORES, 2 * D], bf16, tag="gsb")
            nc.sync.dma_start(out=gsb, in_=gath.rearrange("(j p) x -> p j x", p=D))

            S0b = comb.tile([D, D], bf16, tag="s0b")
            nc.vector.memset(S0b, 0.0)
            for j in range(NCORES - 1):
                mj = maskf_sb[:, j * D:(j + 1) * D]
                # select-based masking: never multiplies NaN by 0
                ah = comb.tile([D, D], bf16, tag="ah")
                nc.vector.tensor_copy(ah, eye128b)
                nc.vector.copy_predicated(ah, mj, gsb[:, j, 0:D])
                cp = comb_ps.tile([D, D], mybir.dt.float32, tag="cp")
                nc.tensor.matmul(cp, lhsT=ah, rhs=S0b, start=True, stop=True)
                bsel = comb.tile([D, D], bf16, tag="bsel")
                nc.vector.memset(bsel, 0.0)
                nc.vector.copy_predicated(bsel, mj, gsb[:, j, D:2 * D])
                S0n = comb.tile([D, D], bf16, tag="s0b")
                nc.vector.tensor_add(S0n, cp, bsel)
                S0b = S0n

            # ---- phase 2: outputs ----
            with tc.tile_pool(name="o_ps", bufs=3, space="PSUM") as o_ps, \
                 tc.tile_pool(name="o_sb", bufs=3) as o_sb:
                for g4 in range(PC // 4):
                    op = o_ps.tile([C, 4 * D], mybir.dt.float32, tag="op")
                    for j in range(4):
                        ci = g4 * 4 + j
                        sl = slice(j * D, (j + 1) * D)
                        nc.tensor.matmul(op[:, sl],
                                         lhsT=Att[:, ci * C:(ci + 1) * C],
                                         rhs=UW[:, ci * 256:ci * 256 + 128],
                                         start=True, stop=False)
                        if ci > 0:
                            nc.tensor.matmul(op[:, sl],
                                             lhsT=Rt[:, ci * C:(ci + 1) * C],
                                             rhs=Ball[:, (ci - 1) * D:ci * D],
                                             start=False, stop=False)
                        nc.tensor.matmul(op[:, sl],
                                         lhsT=Zt[:, ci * C:(ci + 1) * C],
                                         rhs=S0b, start=False, stop=True)
                    osb = o_sb.tile([C, 4 * D], mybir.dt.float32, tag="osb")
                    nc.scalar.copy(osb, op)
                    nc.sync.dma_start(
                        out=out_v[:, g4 * 4:(g4 + 1) * 4, :],
                        in_=osb.rearrange("p (j x) -> p j x", x=D),
                    )
                if dbg:
                    nc.sync.dma_start(out=dbg["dbg_ball"], in_=Ball)
                    nc.sync.dma_start(out=dbg["dbg_zt"], in_=Zt)
                    nc.sync.dma_start(out=dbg["dbg_rt"], in_=Rt)
                    nc.sync.dma_start(out=dbg["dbg_s0"], in_=S0b)
                    gb = comb.tile([D, NCORES * 2 * D], mybir.dt.bfloat16, tag="gb")
                    nc.vector.tensor_copy(gb.rearrange("p (j x) -> p j x", x=2 * D), gsb)
                    nc.sync.dma_start(out=dbg["dbg_gath"].rearrange("(j p) x -> p j x", p=D), in_=gb.rearrange("p (j x) -> p j x", x=2 * D))


def _prep_inputs(q, k, v, beta):
    """Host-side layout prep (elementwise + transpose only): per-core in_maps."""
    kb = k * beta
    vb = v * beta
    in_maps = []
    eye = np.eye(D, dtype=np.float32)
    for c in range(NCORES):
        sl = slice(c * ROWS, (c + 1) * ROWS)
        kc, qc, kbc, vbc = k[sl], q[sl], kb[sl], vb[sl]
        kT = kc.T.reshape(D, PC, C)
        kbT = kbc.T.reshape(D, PC, C)
        qT = qc.T.reshape(D, PC, C)
        kkq = np.concatenate([kT, kbT, qT], axis=2).reshape(D, PC * 192)
        vkn = np.concatenate([vbc, -kbc], axis=1)
        sel = (np.arange(NCORES) < c).astype(np.float32)
        maskf = np.concatenate([np.full((D, D), int(s), np.uint8) for s in sel], axis=1)
        in_maps.append({
            "kkq": np.ascontiguousarray(kkq).astype(BF),
            "knat": np.ascontiguousarray(kc).astype(BF),
            "vkn": np.ascontiguousarray(vkn).astype(BF),
            "maskf": maskf,
        })
    return in_maps


def kernel(q, k, v, beta, chunk_size):
    global LAST_RESULTS
    from concourse.bass_utils import run_bass_kernel_spmd

    q = np.ascontiguousarray(np.asarray(q, dtype=np.float32))
    k = np.ascontiguousarray(np.asarray(k, dtype=np.float32))
    v = np.ascontiguousarray(np.asarray(v, dtype=np.float32))
    beta = np.ascontiguousarray(np.asarray(beta, dtype=np.float32))
    assert int(chunk_size) == C and q.shape == (L, D)

    if "nc" not in _CACHE:
        _CACHE["nc"] = _build_nc()
    nc = _CACHE["nc"]

    in_maps = _prep_inputs(q, k, v, beta)
    trace = bool(int(os.environ.get("KERNEL_TRACE", "0")))
    res = run_bass_kernel_spmd(nc, in_maps, core_ids=list(range(NCORES)),
                               trace=trace)
    LAST_RESULTS = res
    out = np.concatenate([r["out"] for r in res.results], axis=0)
    return out.astype(np.float32)
